# revision 43
# baseline (speedup 1.0000x reference)
"""AttentionPooling (segment softmax + weighted segment sum) on 8 trn2 cores.

Math (per graph g): out[g] = sum_n softmax_g(s)_n * x[n] over nodes n with
batch[n] == g, where s = tanh(x @ W1 + b1) @ W2 + b2.

Key observations:
  * |s| <= ||W2||_1 + |b2| ~= 11.3 (tanh output bounded by 1), so exp(s)
    cannot overflow fp32 -> the segment-max subtraction of the reference is
    unnecessary; we accumulate unnormalized exp(s)*x and exp(s) and divide
    once at the end.
  * batch is sorted, so sharding by graph (128 graphs per core) gives each
    core one contiguous node range: pure data parallel, no collectives.
  * The segment sum is a matmul with a one-hot(weighted) matrix
    S[n, g] = e_n * (batch[n]==g) mapping to TensorE.  Because batch is
    KNOWN AT BUILD TIME and sorted, each 128-node block only touches graphs
    inside one (rarely two) 32-graph windows, so S shrinks to [128, 32] per
    block and the matmul writes a 32-aligned PSUM row window (legal
    tile_position).  That cuts the one-hot build (DVE) 4x and the PE weight
    loads 4x vs a [128, 128] one-hot.
  * TensorE contracts over the partition dim, so the MLP needs x with hidden
    on partitions (xT) while pooling needs nodes on partitions (xaug).
    On-chip transposes cost more than streaming both copies from HBM.
  * Both x streams are fp8 e3m4 (4 mantissa bits, range +-15.5): measured
    rel err 1.46e-2 vs the 2e-2 budget.  W1 stays bf16 so the score noise
    doesn't compound with the pool quantization (fp8 W1 pushed the combined
    error over budget).  HBM traffic: 33 MB/core vs 65 MB at 2x bf16.
"""

import sys
from contextlib import ExitStack

import numpy as np

for _p in ("/opt/trn_rl_repo",):
    if _p not in sys.path:
        sys.path.insert(0, _p)

import ml_dtypes

import concourse.bass as bass
import concourse.bacc as bacc
import concourse.tile as tile
from concourse import mybir

N_NODES = 500_000
HIDDEN = 256
NUM_GRAPHS = 1024
N_CORES = 8
G_LOC = NUM_GRAPHS // N_CORES  # 128 graphs per core == PSUM partition dim
H = HIDDEN // 2  # 128 hidden units in the attention MLP
BLK = 128  # nodes per block (matmul contraction tile)
NBPC = 4  # blocks per chunk
CH = BLK * NBPC  # 512 nodes per compute chunk (one PSUM bank at fp32)
CPS = 4  # compute chunks per DMA super-chunk
SUP = CH * CPS  # 2048 nodes per DMA (~1 MB per stream -> efficient descriptors)
WIN = 32  # pool window: graphs per one-hot / PSUM col group
BF16 = mybir.dt.bfloat16
FP8 = mybir.dt.float8e4
E3M4 = mybir.dt.float8e3  # 4 mantissa bits: x streams (rel err ~3%, max ~15.5)
F32 = mybir.dt.float32

_PROGRAM_CACHE: dict = {}


def build_program(
    n_pad: int, passes: tuple, repeats: int = 1, ablate: str = ""
) -> bass.Bass:
    """passes[blk] = tuple of 32-graph windows the block's pool matmul must
    cover (union across cores; usually 1, occasionally 2).  repeats > 1
    re-runs the whole accumulation loop; numerators and denominators both
    scale by `repeats`, so the output is unchanged -- used for timing."""
    assert n_pad % SUP == 0
    nblk = n_pad // BLK
    nsup = n_pad // SUP
    assert len(passes) == nblk

    # flat pass list [(blk, w, col)] and, per (parity, window), the first and
    # last flat index -- parity ping-pongs the PSUM accumulator per pass
    flat = []
    for blk in range(nblk):
        for w in passes[blk]:
            flat.append((blk, w, len(flat)))
    npass = len(flat)
    first = {}
    last = {}
    for blk, w, idx in flat:
        par = idx % 2
        first.setdefault((par, w), idx)
        last[(par, w)] = idx
    pass_of_blk = {}
    for blk, w, idx in flat:
        pass_of_blk.setdefault(blk, []).append((w, idx))

    nc = bacc.Bacc("TRN2")
    # host-swizzled so each super-chunk DMA reads one contiguous ~8KB run per
    # partition: xaug[s, p, b, f] = [x | 1.0][s*SUP + b*BLK + p, f]
    xaug = nc.dram_tensor(
        "xaug", [nsup, BLK, NBPC * CPS, HIDDEN + 1], E3M4, kind="ExternalInput"
    )
    # xT[s, p, j, n] = x[s*SUP + n, BLK*j + p], fp8: feeds only the score MLP
    xT = nc.dram_tensor("xT", [nsup, BLK, 2, SUP], E3M4, kind="ExternalInput")
    # bcols[p, pass] = batch_local[blk(pass)*BLK + p] - 32*w(pass)  (or <0 pad)
    bcols = nc.dram_tensor("bcols", [BLK, max(npass, 1)], F32, kind="ExternalInput")
    # w1[p, j, h] = W1[BLK*j + p, h]
    w1 = nc.dram_tensor("w1", [BLK, 2, H], BF16, kind="ExternalInput")
    w2 = nc.dram_tensor("w2", [H, 1], BF16, kind="ExternalInput")
    b1 = nc.dram_tensor("b1", [H, 1], F32, kind="ExternalInput")
    b2 = nc.dram_tensor("b2", [BLK, 1], F32, kind="ExternalInput")
    out = nc.dram_tensor("out", [G_LOC, HIDDEN], F32, kind="ExternalOutput")

    with tile.TileContext(nc) as tc, ExitStack() as ctx:
        singles = ctx.enter_context(tc.tile_pool(name="singles", bufs=1))
        xa_pool = ctx.enter_context(tc.tile_pool(name="xa", bufs=3))
        xt_pool = ctx.enter_context(tc.tile_pool(name="xt", bufs=3))
        tt_pool = ctx.enter_context(tc.tile_pool(name="tt", bufs=4))
        st_pool = ctx.enter_context(tc.tile_pool(name="st", bufs=8))
        e_pool = ctx.enter_context(tc.tile_pool(name="e", bufs=4))
        hp_pool = ctx.enter_context(tc.tile_pool(name="hp", bufs=3, space="PSUM"))
        sp_pool = ctx.enter_context(tc.tile_pool(name="sp", bufs=3, space="PSUM"))
        acc_pool = ctx.enter_context(tc.tile_pool(name="acc", bufs=1, space="PSUM"))

        w1_sb = singles.tile([BLK, 2, H], BF16)
        nc.sync.dma_start(out=w1_sb, in_=w1[:, :, :])
        w2_sb = singles.tile([H, 1], BF16)
        nc.sync.dma_start(out=w2_sb, in_=w2[:, :])
        b1_sb = singles.tile([H, 1], F32)
        nc.sync.dma_start(out=b1_sb, in_=b1[:, :])
        b2_sb = singles.tile([BLK, 1], F32)
        nc.sync.dma_start(out=b2_sb, in_=b2[:, :])
        bc_sb = singles.tile([BLK, max(npass, 1)], F32)
        nc.sync.dma_start(out=bc_sb, in_=bcols[:, :])
        iota_sb = singles.tile([BLK, WIN], BF16)
        nc.gpsimd.iota(
            out=iota_sb,
            pattern=[[1, WIN]],
            base=0,
            channel_multiplier=0,
            allow_small_or_imprecise_dtypes=True,
        )

        # two accumulators ping-ponged across passes to break back-to-back
        # PSUM accumulate dependences; summed once at the end
        accs = [
            acc_pool.tile([G_LOC, HIDDEN + 1], F32, tag=f"acc{i}", name=f"acc{i}")
            for i in range(2)
        ]

        def chunk_scores(rep, s, q, tt, xa, xt):
            """Scores + exp for chunk (s, q), emitted one chunk after its MLP
            so the PE (in-order queue) fills the tanh latency with the next
            chunk's MLP matmuls."""
            sp = sp_pool.tile([BLK, NBPC], F32)
            for b in range(NBPC):
                nc.tensor.matmul(
                    sp[:, b : b + 1],
                    lhsT=tt[:, b * BLK : (b + 1) * BLK],
                    rhs=w2_sb,
                    start=True,
                    stop=True,
                )

            ee = e_pool.tile([BLK, NBPC], F32)
            nc.scalar.activation(
                out=ee, in_=sp, func=mybir.ActivationFunctionType.Exp, bias=b2_sb
            )
            return ee

        def chunk_pool(rep, s, q, ee, xa, xt):
            """One-hot pool for chunk (s, q), emitted two chunks after its
            MLP so exp + one-hot builds have a full chunk to complete."""
            for b in range(NBPC):
                blk = (s * CPS + q) * NBPC + b
                pool_rhs = (
                    xa[:, q * NBPC + b, :]
                    if ablate != "no_xaug"
                    else xt[:, 0, 0 : HIDDEN + 1]
                )
                for w, idx in pass_of_blk.get(blk, ()):
                    par = idx % 2
                    st = st_pool.tile([BLK, WIN], BF16, tag="st", name="st")
                    nc.vector.tensor_scalar(
                        out=st,
                        in0=iota_sb,
                        scalar1=bc_sb[:, idx : idx + 1],
                        scalar2=ee[:, b : b + 1],
                        op0=mybir.AluOpType.is_equal,
                        op1=mybir.AluOpType.mult,
                    )
                    nc.tensor.matmul(
                        accs[par][WIN * w : WIN * (w + 1), :],
                        lhsT=st,
                        rhs=pool_rhs,
                        start=(rep == 0 and idx == first[(par, w)]),
                        stop=(rep == repeats - 1 and idx == last[(par, w)]),
                        tile_position=(0, WIN * w),
                    )

        pend_mlp = None  # chunk awaiting scores (lag 1)
        pend_sc = None  # chunk awaiting pool (lag 2)
        for s_iter in range(nsup * repeats):
            rep, s = divmod(s_iter, nsup)
            if ablate != "no_xaug":
                xa = xa_pool.tile([BLK, NBPC * CPS, HIDDEN + 1], E3M4)
                nc.sync.dma_start(out=xa, in_=xaug[s])
            else:
                xa = None
            xt = xt_pool.tile([BLK, 2, SUP], E3M4)
            nc.sync.dma_start(out=xt, in_=xT[s])

            for q in range(CPS):
                if ablate != "no_mlp":
                    hp = hp_pool.tile([H, CH], F32)
                    nc.tensor.matmul(
                        hp,
                        lhsT=w1_sb[:, 0, :],
                        rhs=xt[:, 0, q * CH : (q + 1) * CH],
                        start=True,
                        stop=False,
                    )
                    nc.tensor.matmul(
                        hp,
                        lhsT=w1_sb[:, 1, :],
                        rhs=xt[:, 1, q * CH : (q + 1) * CH],
                        start=False,
                        stop=True,
                    )

                    tt = tt_pool.tile([H, CH], E3M4)
                    nc.scalar.activation(
                        out=tt,
                        in_=hp,
                        func=mybir.ActivationFunctionType.Tanh,
                        bias=b1_sb,
                    )
                else:
                    tt = xt[:, 0, q * CH : (q + 1) * CH]

                if pend_sc is not None:
                    chunk_pool(*pend_sc)
                    pend_sc = None
                if pend_mlp is not None:
                    ee = chunk_scores(*pend_mlp)
                    pend_sc = (*pend_mlp[:3], ee, *pend_mlp[4:])
                pend_mlp = (rep, s, q, tt, xa, xt)
        if pend_sc is not None:
            chunk_pool(*pend_sc)
        ee = chunk_scores(*pend_mlp)
        chunk_pool(*pend_mlp[:3], ee, *pend_mlp[4:])

        # any (parity, window) never touched would leave garbage rows; zero
        # them via a start=True matmul with an all-zero one-hot is not needed
        # because every window has >=2 passes (checked host-side).
        acc1_sb = singles.tile([G_LOC, HIDDEN + 1], F32)
        nc.vector.tensor_copy(out=acc1_sb, in_=accs[1])
        acc = singles.tile([G_LOC, HIDDEN + 1], F32)
        nc.vector.tensor_add(out=acc, in0=accs[0], in1=acc1_sb)
        denom = singles.tile([G_LOC, 1], F32)
        nc.vector.tensor_scalar_max(
            out=denom, in0=acc[:, HIDDEN : HIDDEN + 1], scalar1=1e-30
        )
        rdenom = singles.tile([G_LOC, 1], F32)
        nc.vector.reciprocal(out=rdenom, in_=denom)
        out_sb = singles.tile([G_LOC, HIDDEN], F32)
        nc.vector.tensor_scalar_mul(out=out_sb, in0=acc[:, 0:HIDDEN], scalar1=rdenom)
        nc.sync.dma_start(out=out[:, :], in_=out_sb)

    nc.finalize()
    return nc


def make_in_maps(x, batch, W1, b1, W2, b2):
    """Shard by graph (128 contiguous graphs per core), pad node counts to a
    common multiple of SUP, and lay out the per-core device arrays.  Also
    derives the uniform (across cores) pool pass structure."""
    x = np.asarray(x, dtype=np.float32)
    batch = np.asarray(batch)
    bounds = np.searchsorted(batch, np.arange(0, NUM_GRAPHS + 1, G_LOC))
    n_loc_max = int(np.diff(bounds).max())
    n_pad = max(SUP, ((n_loc_max + SUP - 1) // SUP) * SUP)
    nblk = n_pad // BLK

    # local (per-core) batch ids, -1 padding
    bl_all = np.full((N_CORES, n_pad), -1.0, np.float32)
    for c in range(N_CORES):
        s, e = int(bounds[c]), int(bounds[c + 1])
        bl_all[c, : e - s] = batch[s:e].astype(np.float32) - np.float32(c * G_LOC)

    # uniform pass structure: per block, union of windows over cores
    passes = []
    for blk in range(nblk):
        seg = bl_all[:, blk * BLK : (blk + 1) * BLK]
        ws = sorted({int(g) // WIN for g in np.unique(seg) if g >= 0})
        passes.append(tuple(ws))
    passes = tuple(passes)

    # per (parity, window) pass counts must be >= 1 so start/stop exist
    flat = [(blk, w) for blk in range(nblk) for w in passes[blk]]
    npass = len(flat)
    cnt = {}
    for i, (blk, w) in enumerate(flat):
        cnt[(i % 2, w)] = cnt.get((i % 2, w), 0) + 1
    for w in range(G_LOC // WIN):
        assert cnt.get((0, w), 0) >= 1 and cnt.get((1, w), 0) >= 1, (
            f"window {w} missing a parity; need fallback zeroing"
        )

    # w1[p, j, h] = W1[BLK*j + p, h], bf16 (scores must stay clean: the e3m4
    # pool stream eats most of the error budget)
    w1_8 = np.ascontiguousarray(
        np.asarray(W1, np.float32)
        .astype(ml_dtypes.bfloat16)
        .reshape(2, BLK, H)
        .transpose(1, 0, 2)
    )
    w2_bf = np.asarray(W2, np.float32).reshape(H, 1).astype(ml_dtypes.bfloat16)
    b1_f = np.asarray(b1, np.float32).reshape(H, 1)
    b2_f = np.full((BLK, 1), np.float32(np.asarray(b2).reshape(-1)[0]), np.float32)

    in_maps = []
    for c in range(N_CORES):
        s, e = int(bounds[c]), int(bounds[c + 1])
        nloc = e - s
        xs = x[s:e]
        nsup = n_pad // SUP
        nb = NBPC * CPS
        xa = np.zeros((n_pad, HIDDEN + 1), ml_dtypes.float8_e3m4)
        xa[:nloc, :HIDDEN] = xs.astype(ml_dtypes.float8_e3m4)
        xa[:nloc, HIDDEN] = 1.0
        # [s*SUP + b*BLK + p, f] -> [s, p, b, f]
        xa = np.ascontiguousarray(
            xa.reshape(nsup, nb, BLK, HIDDEN + 1).transpose(0, 2, 1, 3)
        )
        # [s, p, j, n] = x[s*SUP + n, BLK*j + p]
        xT = np.zeros((HIDDEN, n_pad), ml_dtypes.float8_e3m4)
        xT[:, :nloc] = xs.T.astype(ml_dtypes.float8_e3m4)
        xT = np.ascontiguousarray(xT.reshape(2, BLK, nsup, SUP).transpose(2, 1, 0, 3))
        bl = bl_all[c]
        bcols = np.full((BLK, max(npass, 1)), -1e9, np.float32)
        for i, (blk, w) in enumerate(flat):
            bcols[:, i] = bl[blk * BLK : (blk + 1) * BLK] - np.float32(WIN * w)
        in_maps.append(
            {
                "xaug": xa,
                "xT": xT,
                "bcols": np.ascontiguousarray(bcols),
                "w1": w1_8,
                "w2": w2_bf,
                "b1": b1_f,
                "b2": b2_f,
            }
        )
    return in_maps, n_pad, passes


def kernel(x, batch, W1, b1, W2, b2):
    from concourse.bass_utils import run_bass_kernel_spmd

    in_maps, n_pad, passes = make_in_maps(x, batch, W1, b1, W2, b2)
    key = (n_pad, passes)
    nc = _PROGRAM_CACHE.get(key)
    if nc is None:
        nc = build_program(n_pad, passes)
        _PROGRAM_CACHE[key] = nc
    res = run_bass_kernel_spmd(nc, in_maps, list(range(N_CORES)))
    return np.concatenate([res.results[c]["out"] for c in range(N_CORES)], axis=0)
